# revision 1
# baseline (speedup 1.0000x reference)
"""GNN NodeBlock (segment_sum scatter + 2-layer MLP) on 8 Trainium2 cores.

Strategy (per sharding hint: edge/vertex partitioning by receiver range):
 - 2 graphs x 4 cores each; core owns a 12500-node range and all edges
   whose receiver falls in that range.
 - Host buckets edges by 128-node block (98 blocks/core), pads each block
   to a fixed 1408-edge capacity (11 chunks of 128). Rare overflowing
   blocks (>1408 edges, ~0.3% of blocks) are pre-compressed host-side by
   summing duplicate receivers (<=128 distinct rows always fit).
 - Device, per block: build one-hot [edge, node] via iota+is_equal and
   scatter via matmul into agg_T [De, 128] (feature-major), then the MLP
   entirely feature-major: h = relu(W1.T @ [agg; node_T] + b1),
   out_T = W2.T @ h + b2. No transposes on device; host pre-transposes
   node_data and post-transposes the output.
"""
import numpy as np

import concourse.bacc as bacc
import concourse.mybir as mybir
from concourse.tile import TileContext
from concourse.bass_utils import run_bass_kernel_spmd

B, N, E = 2, 50000, 512000
De, Dv, H, Do = 128, 128, 256, 128
NCORES = 8
CPG = 4                    # cores per graph
NPC = N // CPG             # 12500 nodes per core
NB = (NPC + 127) // 128    # 98 blocks per core
CAP = 1408                 # edge capacity per block
NCH = CAP // 128           # 11 chunks of 128 edges
IDOFF = NCH * 128          # ids start at column 1408
PAYW = 1424                # payload row width (1408 feat + 11 ids + 5 pad)

F32 = mybir.dt.float32


def _build_nc():
    nc = bacc.Bacc("TRN2", target_bir_lowering=False)
    payload = nc.dram_tensor("payload", [NB, 128, PAYW], F32, kind="ExternalInput")
    nodes_t = nc.dram_tensor("nodes_t", [NB, 128, 128], F32, kind="ExternalInput")
    w1 = nc.dram_tensor("w1", [128, 512], F32, kind="ExternalInput")   # [p, dk*256+hm*128+j] = W1[dk*128+p, hm*128+j]
    w2 = nc.dram_tensor("w2", [128, 256], F32, kind="ExternalInput")   # [p, hm*128+j] = W2[hm*128+p, j]
    b1 = nc.dram_tensor("b1", [128, 2], F32, kind="ExternalInput")     # [p, hm] = b1[hm*128+p]
    b2 = nc.dram_tensor("b2", [128, 1], F32, kind="ExternalInput")
    out_t = nc.dram_tensor("out_t", [NB, 128, 128], F32, kind="ExternalOutput")

    with TileContext(nc) as tc:
        with tc.tile_pool(name="const", bufs=1) as cp, \
             tc.tile_pool(name="pay", bufs=3) as payp, \
             tc.tile_pool(name="oh", bufs=3) as ohp, \
             tc.tile_pool(name="nod", bufs=3) as nodp, \
             tc.tile_pool(name="sbwork", bufs=3) as wkp, \
             tc.tile_pool(name="outsb", bufs=3) as outp, \
             tc.tile_pool(name="psA", bufs=2, space="PSUM") as psA, \
             tc.tile_pool(name="psH", bufs=2, space="PSUM") as psH, \
             tc.tile_pool(name="psO", bufs=2, space="PSUM") as psO:
            w1_sb = cp.tile([128, 512], F32)
            nc.sync.dma_start(out=w1_sb[:], in_=w1[:, :])
            w2_sb = cp.tile([128, 256], F32)
            nc.sync.dma_start(out=w2_sb[:], in_=w2[:, :])
            b1_sb = cp.tile([128, 2], F32)
            nc.sync.dma_start(out=b1_sb[:], in_=b1[:, :])
            b2_sb = cp.tile([128, 1], F32)
            nc.sync.dma_start(out=b2_sb[:], in_=b2[:, :])

            iota_i = cp.tile([128, 128], mybir.dt.int32)
            nc.gpsimd.iota(iota_i[:], pattern=[[1, 128]], base=0, channel_multiplier=0)
            iota_f = cp.tile([128, 128], F32)
            nc.vector.tensor_copy(iota_f[:], iota_i[:])

            for b in range(NB):
                pay = payp.tile([128, PAYW], F32)
                nc.sync.dma_start(out=pay[:], in_=payload[b])
                nod = nodp.tile([128, 128], F32)
                nc.sync.dma_start(out=nod[:], in_=nodes_t[b])

                oh = ohp.tile([128, NCH * 128], F32)
                for c in range(NCH):
                    nc.vector.tensor_tensor(
                        out=oh[:, c * 128:(c + 1) * 128],
                        in0=pay[:, IDOFF + c:IDOFF + c + 1].to_broadcast([128, 128]),
                        in1=iota_f[:],
                        op=mybir.AluOpType.is_equal,
                    )

                agg = psA.tile([128, 128], F32, space="PSUM")
                for c in range(NCH):
                    nc.tensor.matmul(
                        out=agg[:],
                        lhsT=pay[:, c * 128:(c + 1) * 128],
                        rhs=oh[:, c * 128:(c + 1) * 128],
                        start=(c == 0),
                        stop=(c == NCH - 1),
                    )
                agg_sb = wkp.tile([128, 128], F32, tag="aggsb")
                nc.scalar.copy(agg_sb[:], agg[:])

                hps = psH.tile([128, 256], F32, space="PSUM")
                for hm in range(2):
                    for dk in range(2):
                        nc.tensor.matmul(
                            out=hps[:, hm * 128:(hm + 1) * 128],
                            lhsT=w1_sb[:, dk * 256 + hm * 128:dk * 256 + (hm + 1) * 128],
                            rhs=(agg_sb[:] if dk == 0 else nod[:]),
                            start=(dk == 0),
                            stop=(dk == 1),
                        )
                h_sb = wkp.tile([128, 256], F32, tag="hsb")
                for hm in range(2):
                    nc.scalar.activation(
                        out=h_sb[:, hm * 128:(hm + 1) * 128],
                        in_=hps[:, hm * 128:(hm + 1) * 128],
                        func=mybir.ActivationFunctionType.Relu,
                        bias=b1_sb[:, hm:hm + 1],
                    )

                ops = psO.tile([128, 128], F32, space="PSUM")
                for hm in range(2):
                    nc.tensor.matmul(
                        out=ops[:],
                        lhsT=w2_sb[:, hm * 128:(hm + 1) * 128],
                        rhs=h_sb[:, hm * 128:(hm + 1) * 128],
                        start=(hm == 0),
                        stop=(hm == 1),
                    )
                o_sb = outp.tile([128, 128], F32)
                nc.scalar.activation(
                    out=o_sb[:],
                    in_=ops[:],
                    func=mybir.ActivationFunctionType.Identity,
                    bias=b2_sb[:, 0:1],
                )
                nc.sync.dma_start(out=out_t[b], in_=o_sb[:])
    nc.compile()
    return nc


def _prep_core(efeat, blk, within, node_block_t):
    """Build one core's payload from its edges.

    efeat: [ne, De] f32 edge features routed to this core
    blk:   [ne] int block index (0..NB-1)
    within:[ne] int node index within block (0..127)
    node_block_t: [NB, 128, 128] f32 transposed node data
    """
    counts = np.bincount(blk, minlength=NB)
    if (counts > CAP).any():
        # compress overflowing blocks: sum duplicate receivers (<=128 rows)
        keep = np.ones(len(blk), bool)
        extra_f, extra_b, extra_w = [], [], []
        for ob in np.nonzero(counts > CAP)[0]:
            sel = blk == ob
            keep[sel] = False
            seg = np.zeros((128, De), np.float32)
            np.add.at(seg, within[sel], efeat[sel])
            rows = np.unique(within[sel])
            extra_f.append(seg[rows])
            extra_b.append(np.full(len(rows), ob, blk.dtype))
            extra_w.append(rows.astype(within.dtype))
        efeat = np.concatenate([efeat[keep]] + extra_f)
        blk = np.concatenate([blk[keep]] + extra_b)
        within = np.concatenate([within[keep]] + extra_w)
        counts = np.bincount(blk, minlength=NB)

    order = np.argsort(blk, kind="stable")
    blk_s = blk[order]
    offs = np.zeros(NB, np.int64)
    np.cumsum(counts[:-1], out=offs[1:])
    pos = np.arange(len(blk_s)) - offs[blk_s]
    c = pos // 128
    prow = pos - c * 128

    payload = np.zeros((NB, 128, PAYW), np.float32)
    feat_view = payload[:, :, :IDOFF].reshape(NB, 128, NCH, 128)
    feat_view[blk_s, prow, c, :] = efeat[order]
    id_view = payload[:, :, IDOFF:IDOFF + NCH]
    id_view[:] = 128.0  # padding id -> matches no iota column
    id_view[blk_s, prow, c] = within[order].astype(np.float32)
    return {"payload": payload, "nodes_t": node_block_t}


def kernel(edge_data, node_data, W1, b1, W2, b2, receiver_ids, _trace=False):
    edge_data = np.asarray(edge_data, np.float32)
    node_data = np.asarray(node_data, np.float32)
    W1 = np.asarray(W1, np.float32)
    b1 = np.asarray(b1, np.float32)
    W2 = np.asarray(W2, np.float32)
    b2 = np.asarray(b2, np.float32)
    rid = np.asarray(receiver_ids).astype(np.int64)

    w1_dev = np.ascontiguousarray(
        W1.reshape(2, 128, H).transpose(1, 0, 2).reshape(128, 2 * H))
    w2_dev = np.ascontiguousarray(
        W2.reshape(2, 128, Do).transpose(1, 0, 2).reshape(128, 2 * Do))
    b1_dev = np.ascontiguousarray(b1.reshape(2, 128).T)
    b2_dev = np.ascontiguousarray(b2.reshape(128, 1))

    in_maps = []
    for core in range(NCORES):
        g, part = divmod(core, CPG)
        base = part * NPC
        sel = (rid[g] >= base) & (rid[g] < base + NPC)
        local = rid[g][sel] - base
        blk = local // 128
        within = local - blk * 128
        efeat = edge_data[g][sel]

        nb_pad = NB * 128
        nd = np.zeros((nb_pad, Dv), np.float32)
        nd[:NPC] = node_data[g, base:base + NPC]
        node_block_t = np.ascontiguousarray(
            nd.reshape(NB, 128, Dv).transpose(0, 2, 1))

        m = _prep_core(efeat, blk, within, node_block_t)
        m.update({"w1": w1_dev, "w2": w2_dev, "b1": b1_dev, "b2": b2_dev})
        in_maps.append(m)

    nc = _build_nc()
    res = run_bass_kernel_spmd(nc, in_maps, core_ids=list(range(NCORES)),
                               trace=_trace)

    out = np.empty((B, N, Do), np.float32)
    for core in range(NCORES):
        g, part = divmod(core, CPG)
        ot = res.results[core]["out_t"]            # [NB, 128 o, 128 j]
        on = ot.transpose(0, 2, 1).reshape(NB * 128, Do)
        out[g, part * NPC:(part + 1) * NPC] = on[:NPC]
    if _trace:
        kernel._last = res
    return out


# revision 2
# speedup vs baseline: 2.2213x; 2.2213x over previous
"""GNN NodeBlock (segment_sum scatter + 2-layer MLP) on 8 Trainium2 cores.

Strategy (per sharding hint: edge/vertex partitioning by receiver range):
 - 2 graphs x 4 cores each; core owns a 12500-node range and all edges
   whose receiver falls in that range.
 - Host buckets edges by 128-node block (98 blocks/core), pads each block
   to a fixed 1408-edge capacity (11 chunks of 128). Rare overflowing
   blocks (>1408 edges) are pre-compressed host-side by summing duplicate
   receivers (<=128 distinct rows always fit).
 - Device, per block: build one-hot [edge, node] via iota+is_equal (one
   wide DVE op) and scatter via matmul into agg_T [De, 128]
   (feature-major), then the MLP feature-major and batched over groups of
   4 blocks: h = relu(W1.T @ [agg; node_T] + b1), out_T = W2.T @ h + b2.
   No device transposes; host pre-transposes node_data and
   post-transposes the output.
 - Compute in bf16 (inputs cast host-side; PSUM accumulation is fp32).
"""
import os
import numpy as np
import ml_dtypes

import concourse.bacc as bacc
import concourse.mybir as mybir
from concourse.tile import TileContext
from concourse.bass_utils import run_bass_kernel_spmd

B, N, E = 2, 50000, 512000
De, Dv, H, Do = 128, 128, 256, 128
NCORES = 8
CPG = 4                    # cores per graph
NPC = N // CPG             # 12500 nodes per core
NB = (NPC + 127) // 128    # 98 blocks per core
CAP = 1408                 # edge capacity per block
NCH = CAP // 128           # 11 chunks of 128 edges
IDOFF = NCH * 128          # ids start at column 1408
GRP = 4                    # blocks per MLP group
GROUPS = [GRP] * (NB // GRP) + ([NB % GRP] if NB % GRP else [])

USE_BF16 = os.environ.get("GNN_FP32", "") != "1"
F32 = mybir.dt.float32
if USE_BF16:
    CDT, NPDT, PAYW = mybir.dt.bfloat16, ml_dtypes.bfloat16, 1440
else:
    CDT, NPDT, PAYW = F32, np.float32, 1424


def _build_nc():
    nc = bacc.Bacc("TRN2", target_bir_lowering=False)
    payload = nc.dram_tensor("payload", [NB, 128, PAYW], CDT, kind="ExternalInput")
    nodes_g = nc.dram_tensor("nodes_g", [len(GROUPS), 128, GRP * 128], CDT, kind="ExternalInput")
    w1 = nc.dram_tensor("w1", [128, 512], CDT, kind="ExternalInput")   # [p, dk*256+hm*128+j] = W1[dk*128+p, hm*128+j]
    w2 = nc.dram_tensor("w2", [128, 256], CDT, kind="ExternalInput")   # [p, hm*128+j] = W2[hm*128+p, j]
    b1 = nc.dram_tensor("b1", [128, 2], F32, kind="ExternalInput")     # [p, hm] = b1[hm*128+p]
    b2 = nc.dram_tensor("b2", [128, 1], F32, kind="ExternalInput")
    out_g = nc.dram_tensor("out_g", [len(GROUPS), 128, GRP * 128], F32, kind="ExternalOutput")

    with TileContext(nc) as tc:
        with tc.tile_pool(name="const", bufs=1) as cp, \
             tc.tile_pool(name="pay", bufs=6) as payp, \
             tc.tile_pool(name="oh", bufs=6) as ohp, \
             tc.tile_pool(name="nod", bufs=3) as nodp, \
             tc.tile_pool(name="agg4", bufs=3) as aggp, \
             tc.tile_pool(name="hsb", bufs=3) as hp, \
             tc.tile_pool(name="osb", bufs=3) as op_, \
             tc.tile_pool(name="psA", bufs=2, space="PSUM") as psA, \
             tc.tile_pool(name="psH", bufs=2, space="PSUM") as psH, \
             tc.tile_pool(name="psO", bufs=2, space="PSUM") as psO:
            w1_sb = cp.tile([128, 512], CDT)
            nc.sync.dma_start(out=w1_sb[:], in_=w1[:, :])
            w2_sb = cp.tile([128, 256], CDT)
            nc.sync.dma_start(out=w2_sb[:], in_=w2[:, :])
            b1_sb = cp.tile([128, 2], F32)
            nc.sync.dma_start(out=b1_sb[:], in_=b1[:, :])
            b2_sb = cp.tile([128, 1], F32)
            nc.sync.dma_start(out=b2_sb[:], in_=b2[:, :])

            iota_i = cp.tile([128, 128], mybir.dt.int32)
            nc.gpsimd.iota(iota_i[:], pattern=[[1, 128]], base=0, channel_multiplier=0)
            iota_c = cp.tile([128, 128], CDT)
            nc.vector.tensor_copy(iota_c[:], iota_i[:])

            for gi, g_sz in enumerate(GROUPS):
                b0 = gi * GRP
                nod = nodp.tile([128, GRP * 128], CDT)
                nc.sync.dma_start(out=nod[:, :g_sz * 128],
                                  in_=nodes_g[gi, :, :g_sz * 128])
                agg_ps = psA.tile([128, GRP * 128], F32, space="PSUM")

                pays = []
                for g in range(g_sz):
                    b = b0 + g
                    pay = payp.tile([128, PAYW], CDT)
                    nc.sync.dma_start(out=pay[:], in_=payload[b])
                    pays.append(pay)

                    oh = ohp.tile([128, NCH * 128], CDT)
                    nc.vector.tensor_tensor(
                        out=oh[:].rearrange("p (c n) -> p c n", c=NCH),
                        in0=pay[:, IDOFF:IDOFF + NCH].to_broadcast([128, NCH, 128]),
                        in1=iota_c[:, None, :].to_broadcast([128, NCH, 128]),
                        op=mybir.AluOpType.is_equal,
                    )
                    for c in range(NCH):
                        nc.tensor.matmul(
                            out=agg_ps[:, g * 128:(g + 1) * 128],
                            lhsT=pay[:, c * 128:(c + 1) * 128],
                            rhs=oh[:, c * 128:(c + 1) * 128],
                            start=(c == 0),
                            stop=(c == NCH - 1),
                        )

                agg_sb = aggp.tile([128, GRP * 128], CDT)
                nc.scalar.copy(agg_sb[:, :g_sz * 128], agg_ps[:, :g_sz * 128])

                hps = psH.tile([128, 2 * GRP * 128], F32, space="PSUM")
                for hm in range(2):
                    for dk in range(2):
                        nc.tensor.matmul(
                            out=hps[:, hm * GRP * 128:hm * GRP * 128 + g_sz * 128],
                            lhsT=w1_sb[:, dk * 256 + hm * 128:dk * 256 + (hm + 1) * 128],
                            rhs=(agg_sb[:, :g_sz * 128] if dk == 0 else nod[:, :g_sz * 128]),
                            start=(dk == 0),
                            stop=(dk == 1),
                        )
                h_sb = hp.tile([128, 2 * GRP * 128], CDT)
                for hm in range(2):
                    nc.scalar.activation(
                        out=h_sb[:, hm * GRP * 128:hm * GRP * 128 + g_sz * 128],
                        in_=hps[:, hm * GRP * 128:hm * GRP * 128 + g_sz * 128],
                        func=mybir.ActivationFunctionType.Relu,
                        bias=b1_sb[:, hm:hm + 1],
                    )

                ops = psO.tile([128, GRP * 128], F32, space="PSUM")
                for hm in range(2):
                    nc.tensor.matmul(
                        out=ops[:, :g_sz * 128],
                        lhsT=w2_sb[:, hm * 128:(hm + 1) * 128],
                        rhs=h_sb[:, hm * GRP * 128:hm * GRP * 128 + g_sz * 128],
                        start=(hm == 0),
                        stop=(hm == 1),
                    )
                o_sb = op_.tile([128, GRP * 128], F32)
                nc.scalar.activation(
                    out=o_sb[:, :g_sz * 128],
                    in_=ops[:, :g_sz * 128],
                    func=mybir.ActivationFunctionType.Identity,
                    bias=b2_sb[:, 0:1],
                )
                nc.sync.dma_start(out=out_g[gi, :, :g_sz * 128],
                                  in_=o_sb[:, :g_sz * 128])
    nc.compile()
    return nc


def _prep_core(efeat, blk, within, nodes_g_core):
    """Build one core's payload from its edges (already cast to NPDT)."""
    counts = np.bincount(blk, minlength=NB)
    if (counts > CAP).any():
        # compress overflowing blocks: sum duplicate receivers (<=128 rows)
        keep = np.ones(len(blk), bool)
        extra_f, extra_b, extra_w = [], [], []
        for ob in np.nonzero(counts > CAP)[0]:
            sel = blk == ob
            keep[sel] = False
            seg = np.zeros((128, De), np.float32)
            np.add.at(seg, within[sel], efeat[sel].astype(np.float32))
            rows = np.unique(within[sel])
            extra_f.append(seg[rows].astype(efeat.dtype))
            extra_b.append(np.full(len(rows), ob, blk.dtype))
            extra_w.append(rows.astype(within.dtype))
        efeat = np.concatenate([efeat[keep]] + extra_f)
        blk = np.concatenate([blk[keep]] + extra_b)
        within = np.concatenate([within[keep]] + extra_w)
        counts = np.bincount(blk, minlength=NB)

    order = np.argsort(blk, kind="stable")
    blk_s = blk[order]
    offs = np.zeros(NB, np.int64)
    np.cumsum(counts[:-1], out=offs[1:])
    pos = np.arange(len(blk_s)) - offs[blk_s]
    c = pos // 128
    prow = pos - c * 128

    payload = np.zeros((NB, 128, PAYW), NPDT)
    feat_view = payload[:, :, :IDOFF].reshape(NB, 128, NCH, 128)
    feat_view[blk_s, prow, c, :] = efeat[order]
    id_view = payload[:, :, IDOFF:IDOFF + NCH]
    id_view[:] = NPDT(128.0)  # padding id -> matches no iota column
    id_view[blk_s, prow, c] = within[order].astype(NPDT)
    return {"payload": payload, "nodes_g": nodes_g_core}


def kernel(edge_data, node_data, W1, b1, W2, b2, receiver_ids, _trace=False):
    edge_data = np.asarray(edge_data, np.float32)
    node_data = np.asarray(node_data, np.float32)
    W1 = np.asarray(W1, np.float32)
    b1 = np.asarray(b1, np.float32)
    W2 = np.asarray(W2, np.float32)
    b2 = np.asarray(b2, np.float32)
    rid = np.asarray(receiver_ids).astype(np.int64)

    w1_dev = np.ascontiguousarray(
        W1.reshape(2, 128, H).transpose(1, 0, 2).reshape(128, 2 * H)).astype(NPDT)
    w2_dev = np.ascontiguousarray(
        W2.reshape(2, 128, Do).transpose(1, 0, 2).reshape(128, 2 * Do)).astype(NPDT)
    b1_dev = np.ascontiguousarray(b1.reshape(2, 128).T)
    b2_dev = np.ascontiguousarray(b2.reshape(128, 1))

    ng = len(GROUPS)
    in_maps = []
    for core in range(NCORES):
        g, part = divmod(core, CPG)
        base = part * NPC
        sel = (rid[g] >= base) & (rid[g] < base + NPC)
        local = rid[g][sel] - base
        blk = local // 128
        within = local - blk * 128
        efeat = edge_data[g][sel].astype(NPDT)

        nd = np.zeros((ng * GRP * 128, Dv), np.float32)
        nd[:NPC] = node_data[g, base:base + NPC]
        # [ng, 128 d, GRP*128 n]: group-contiguous, feature-major
        nodes_g_core = np.ascontiguousarray(
            nd.reshape(ng, GRP * 128, Dv).transpose(0, 2, 1)).astype(NPDT)

        m = _prep_core(efeat, blk, within, nodes_g_core)
        m.update({"w1": w1_dev, "w2": w2_dev, "b1": b1_dev, "b2": b2_dev})
        in_maps.append(m)

    nc = _build_nc()
    res = run_bass_kernel_spmd(nc, in_maps, core_ids=list(range(NCORES)),
                               trace=_trace)

    out = np.empty((B, N, Do), np.float32)
    for core in range(NCORES):
        g, part = divmod(core, CPG)
        og = res.results[core]["out_g"]                   # [ng, 128 o, GRP*128 j]
        on = og.transpose(0, 2, 1).reshape(ng * GRP * 128, Do)
        out[g, part * NPC:(part + 1) * NPC] = on[:NPC]
    if _trace:
        kernel._last = res
    return out
